# revision 18
# baseline (speedup 1.0000x reference)
"""Trainium2 Bass kernel for nn_MultiHeadAttention_79508434583676.

Reference semantics (faithful to source bugs):
  proj = x @ Wq.T + bq  for x in {Q, K, V}   (Wq projects all three)
  q,k,v = split_heads(proj)                  [B,H,N,dk]
  scores = q @ k.T / sqrt(dk)                [B,H,N,N]
  probs = softmax(scores, axis=1)            (softmax over the HEADS axis)
  A = probs @ v -> combine heads -> A @ Wo.T + bo

Sharding: 8 cores = 4 batches x 2 query-halves. Softmax over heads is local
to each (n,m) score position, so with all heads on one core there is no
cross-core coupling -> no collectives. K/V work for a batch is duplicated
across its 2 cores.

Per-core pipeline (NQ=1024 query rows, NK=2048 key rows, D=512, H=8, dk=64):
  phase 1: PE-transpose Q,K,V tiles (bf16); bf16 projections using host-
           pre-transposed Wq.T; q,k projections kept transposed [e, n];
           v projection kept natural [m, e]. The k-path weights are host
           pre-scaled by 1/sqrt(dk) so exp runs with scale=1.
  phase 2: 2-deep software pipeline over (m-tile 128, n-chunk 512) steps:
           step c runs row-packed score matmuls + ACT exp for step c; the
           cross-head sum for step c-1 as a bf16 add tree split between
           GpSimd (level 1a) and DVE (1b, 2, 3) + reciprocal_approx_fast
           (DVE) + cast (GpSimd) + the broadcast normalize mul split
           6/8 heads DVE, 2/8 GpSimd; and the col-packed A^T accumulation
           matmuls for step c-2 emitted AFTER the scores so the PE queue
           never head-of-line blocks the exp relay. The PE does no
           reduction work -- engines are balanced at ~5us/step each.
           Output projection consumes A^T PSUM tiles directly; + bo; DMA.
"""

import sys

sys.path.insert(0, "/opt/trn_rl_repo")

import math
from contextlib import ExitStack

import numpy as np

import concourse.bass as bass
from concourse.bacc import Bacc
import concourse.mybir as mybir
import concourse.tile as tile
from concourse.masks import make_identity

F32 = mybir.dt.float32
F32R = mybir.dt.float32r
BF16 = mybir.dt.bfloat16
ADD = mybir.AluOpType.add
MULT = mybir.AluOpType.mult

B, N, D, H = 4, 2048, 512, 8
DK = D // H           # 64
NQ = N // 2           # 1024 query rows per core
NK = N                # 2048 key rows per core
NCH = 512             # n-chunk (score matmul free dim)
N_CHUNKS = NQ // NCH  # 2
MT = NK // 128        # 16 m-tiles
ET = D // 128         # 4 e-tiles (= head pairs)
SCALE = 1.0 / math.sqrt(DK)


def r32(ap):
    return ap.bitcast(F32R)


def build_nc(repeat: int | None = None) -> bass.Bass:
    nc = Bacc()

    Qd = nc.dram_tensor("q_in", [NQ, D], F32, kind="ExternalInput")
    Kd = nc.dram_tensor("k_in", [NK, D], F32, kind="ExternalInput")
    Vd = nc.dram_tensor("v_in", [NK, D], F32, kind="ExternalInput")
    WqTd = nc.dram_tensor("wqt", [D, D], F32, kind="ExternalInput")  # Wq.T [d, e]
    WqTsd = nc.dram_tensor("wqts", [D, D], F32, kind="ExternalInput")  # Wq.T/sqrt(dk)
    WoTd = nc.dram_tensor("wot", [D, D], F32, kind="ExternalInput")  # Wo.T [e, eo]
    bqd = nc.dram_tensor("bq", [1, D], F32, kind="ExternalInput")
    bqsd = nc.dram_tensor("bqs", [1, D], F32, kind="ExternalInput")  # bq/sqrt(dk)
    bod = nc.dram_tensor("bo", [1, D], F32, kind="ExternalInput")
    OUT = nc.dram_tensor("out", [NQ, D], F32, kind="ExternalOutput")

    with ExitStack() as ctx:
        tc = ctx.enter_context(tile.TileContext(nc))
        _emit(ctx, tc, Qd, Kd, Vd, WqTd, WqTsd, WoTd, bqd, bqsd, bod, OUT,
              repeat=repeat)

    nc.finalize()
    return nc


def _emit(ctx, tc, Qd, Kd, Vd, WqTd, WqTsd, WoTd, bqd, bqsd, bod, OUT,
          repeat=None):
    nc = tc.nc

    # ---------------------------------------------------------- constants
    const_pool = ctx.enter_context(tc.tile_pool(name="const", bufs=1))

    ident = const_pool.tile([128, 128], F32, name="ident")
    make_identity(nc, ident)
    ident_bf = const_pool.tile([128, 128], BF16, name="ident_bf")
    make_identity(nc, ident_bf)

    # bq with e on partitions: element (p, t) = bq[t*128 + p]
    bq_cols = const_pool.tile([128, ET], F32, name="bq_cols")
    nc.sync.dma_start(bq_cols[:, :], bqd[0, :].rearrange("(t p) -> p t", p=128))
    bqs_cols = const_pool.tile([128, ET], F32, name="bqs_cols")
    nc.sync.dma_start(bqs_cols[:, :], bqsd[0, :].rearrange("(t p) -> p t", p=128))

    bq_bcast = const_pool.tile([128, D], F32, name="bq_bcast")
    bo_bcast = const_pool.tile([128, D], F32, name="bo_bcast")

    wqt_bf = []   # Wq.T bf16 tiles, d on partitions (q/v path)
    wqts_bf = []  # Wq.T/sqrt(dk) bf16 tiles (k path)
    wot_bf = []   # Wo.T bf16 tiles, e on partitions
    for t in range(ET):
        wqt_bf.append(
            const_pool.tile([128, D], BF16, name=f"wqtb{t}", tag=f"wqtb{t}")
        )
        wqts_bf.append(
            const_pool.tile([128, D], BF16, name=f"wqtsb{t}", tag=f"wqtsb{t}")
        )
        wot_bf.append(
            const_pool.tile([128, D], BF16, name=f"wotb{t}", tag=f"wotb{t}")
        )

    with tc.tile_pool(name="setup_stage", bufs=2) as sstage:
        for bias_d, dst in ((bqd, bq_bcast), (bod, bo_bcast)):
            nc.sync.dma_start(dst[:, :], bias_d[0, :].partition_broadcast(128))
        for src_d, dsts in ((WqTd, wqt_bf), (WqTsd, wqts_bf), (WoTd, wot_bf)):
            for t in range(ET):
                wstage = sstage.tile([128, D], F32, name="wstage", tag="wstage")
                nc.sync.dma_start(wstage[:, :], src_d[t * 128 : (t + 1) * 128, :])
                nc.vector.tensor_copy(dsts[t][:, :], wstage[:, :])

    # --------------------------------------------------- persistent SBUF
    qp_pool = ctx.enter_context(tc.tile_pool(name="qp", bufs=ET))
    kp_pool = ctx.enter_context(tc.tile_pool(name="kp", bufs=ET))
    vp_pool = ctx.enter_context(tc.tile_pool(name="vp", bufs=MT))
    qpT = [qp_pool.tile([128, NQ], BF16, name=f"qpT{t}", tag="qpT") for t in range(ET)]
    kpT = [kp_pool.tile([128, NK], BF16, name=f"kpT{t}", tag="kpT") for t in range(ET)]
    vp = [vp_pool.tile([128, D], BF16, name=f"vp{m}", tag="vp") for m in range(MT)]

    # ----------------------------------------------------------- phase 1
    def load_transpose(stage_pool, ps_pool, Xd, xT_all, n_rows):
        """DMA [n_rows, D] fp32 from DRAM, PE-transpose (bf16) into a single
        [128, ET*n_rows] tensor (d-tile-major along free); one scatter-copy
        evacuation per 128-row block."""
        xT3 = xT_all[:, :].rearrange("p (t n) -> p t n", t=ET)
        for ntile in range(n_rows // 128):
            st = stage_pool.tile([128, D], F32, name="x_stage", tag="stage")
            dma_eng = nc.sync if ntile % 2 == 0 else nc.scalar
            dma_eng.dma_start(st[:, :], Xd[ntile * 128 : (ntile + 1) * 128, :])
            st_bf = stage_pool.tile([128, D], BF16, name="x_stage_bf", tag="stage_bf")
            nc.scalar.copy(st_bf[:, :], st[:, :])
            ps = ps_pool.tile([128, D], BF16, name="ps_tr", tag="ps_s")
            for dt_ in range(ET):
                nc.tensor.transpose(
                    ps[:, dt_ * 128 : (dt_ + 1) * 128],
                    st_bf[:, dt_ * 128 : (dt_ + 1) * 128],
                    ident_bf[:, :],
                )
            nc.vector.tensor_copy(
                xT3[:, :, ntile * 128 : (ntile + 1) * 128],
                ps[:, :].rearrange("p (t n) -> p t n", t=ET),
            )

    def project_T(ps_pool, xT_all, xpT, n_rows, w_bf, b_cols):
        """xpT[et][e, n] = sum_d W[d, e] xT[d, n] + b[e]  (bf16)."""
        for et in range(ET):
            for nch in range(n_rows // NCH):
                ps = ps_pool.tile([128, NCH], F32, name="ps_proj", tag="psA")
                for dt_ in range(ET):
                    base = dt_ * n_rows + nch * NCH
                    nc.tensor.matmul(
                        ps[:, :],
                        w_bf[dt_][:, et * 128 : (et + 1) * 128],
                        xT_all[:, base : base + NCH],
                        start=(dt_ == 0),
                        stop=(dt_ == ET - 1),
                    )
                nc.vector.tensor_scalar_add(
                    xpT[et][:, nch * NCH : (nch + 1) * NCH],
                    ps[:, :],
                    b_cols[:, et : et + 1],
                )

    stage_pool = ctx.enter_context(tc.tile_pool(name="stage", bufs=4))
    xtq_pool = ctx.enter_context(tc.tile_pool(name="xtq", bufs=1))
    xtk_pool = ctx.enter_context(tc.tile_pool(name="xtk", bufs=1))
    xtv_pool = ctx.enter_context(tc.tile_pool(name="xtv", bufs=1))
    e_pool = ctx.enter_context(tc.tile_pool(name="ework", bufs=5))
    p_pool = ctx.enter_context(tc.tile_pool(name="pwork", bufs=4))
    r_pool = ctx.enter_context(tc.tile_pool(name="rwork", bufs=2))
    a_pool = ctx.enter_context(tc.tile_pool(name="abuf", bufs=2 * ET))
    o_pool = ctx.enter_context(tc.tile_pool(name="ostage", bufs=2))
    # PSUM: 8 banks total. ps_s pool: 2 slots x [128,1024]f32 (2 banks each);
    # phase-1 transposes + out-proj share the slots via the same tag.
    # ps_a pool: 4 slots x 1 bank; phase-1 projections share via tag.
    ps_s_pool = ctx.enter_context(tc.tile_pool(name="ps_s", bufs=2, space="PSUM"))
    ps_a_pool = ctx.enter_context(tc.tile_pool(name="ps_a", bufs=ET, space="PSUM"))

    ps_t_pool = ps_s_pool
    ps_p_pool = ps_a_pool

    def body():
        qT = xtq_pool.tile([128, ET * NQ], BF16, name="qT", tag="qT")
        load_transpose(stage_pool, ps_t_pool, Qd, qT, NQ)
        project_T(ps_p_pool, qT, qpT, NQ, wqt_bf, bq_cols)

        kT = xtk_pool.tile([128, ET * NK], BF16, name="kT", tag="kT")
        load_transpose(stage_pool, ps_t_pool, Kd, kT, NK)
        project_T(ps_p_pool, kT, kpT, NK, wqts_bf, bqs_cols)

        vT = xtv_pool.tile([128, ET * NK], BF16, name="vT", tag="vT")
        load_transpose(stage_pool, ps_t_pool, Vd, vT, NK)
        # vp[m][p, e] = sum_d vT[d, m*128+p] wqt_bf[d, e] + bq[e]
        for m in range(MT):
            ps = ps_p_pool.tile([128, D], F32, name="ps_vp", tag="psA")
            for dt_ in range(ET):
                nc.tensor.matmul(
                    ps[:, :],
                    vT[:, dt_ * NK + m * 128 : dt_ * NK + (m + 1) * 128],
                    wqt_bf[dt_][:, :],
                    start=(dt_ == 0),
                    stop=(dt_ == ET - 1),
                )
            nc.vector.tensor_tensor(vp[m][:, :], ps[:, :], bq_bcast[:, :], ADD)

        # ------------------------------------------------------- phase 2
        def emit_A(psA, mt, P):
            # A^T accumulation, col-packed head pairs
            for pair in range(ET):
                for half in range(2):
                    h = 2 * pair + half
                    nc.tensor.matmul(
                        psA[pair][64 * half : 64 * (half + 1), :],
                        vp[mt][:, h * DK : (h + 1) * DK],
                        P[:, h * NCH : (h + 1) * NCH],
                        start=(mt == 0),
                        stop=(mt == MT - 1),
                        tile_position=(0, 64 * half),
                        # the sim's zero-region tracker can't see the
                        # partition offset; the two col-packed halves of
                        # one bank are distinct accumulation groups
                        skip_group_check=True,
                    )

        def emit_scores_exp(nch, mt):
            """Score matmuls + exp for one (m-tile, n-chunk); returns E.
            1/sqrt(dk) is pre-folded into the k projection."""
            nsl = slice(nch * NCH, (nch + 1) * NCH)
            msl = slice(mt * 128, (mt + 1) * 128)
            E = e_pool.tile([128, H * NCH], BF16, name="E", tag="E")
            for pair in range(ET):
                # heads 2*pair (partitions 0:64 of e-tile) and 2*pair+1
                # (64:128); the two matmuls row-pack on the PE
                ps_s = ps_s_pool.tile([128, 2 * NCH], F32, name="ps_s", tag="ps_s")
                for half in range(2):
                    hsl = slice(64 * half, 64 * (half + 1))
                    nc.tensor.matmul(
                        ps_s[:, half * NCH : (half + 1) * NCH],
                        kpT[pair][hsl, msl],
                        qpT[pair][hsl, nsl],
                        tile_position=(64 * half, 0),
                    )
                nc.scalar.activation(
                    E[:, pair * 2 * NCH : (pair + 1) * 2 * NCH],
                    ps_s[:, :],
                    mybir.ActivationFunctionType.Exp,
                )
            return E

        def emit_norm(E):
            """Cross-head sum via chained SBUF->SBUF DMA accumulates
            (gpsimd SWDGE desc-gen; the adds run on idle DMA engines),
            one DVE merge add, reciprocal + cast + the broadcast
            normalize mul on DVE. No PE work. Returns P."""
            # T = (b0+b2+b4+b6 | b1+b3+b5+b7)  [128, 2*NCH] bf16
            T = r_pool.tile([128, 2 * NCH], BF16, name="Tsum", tag="Tsum")
            nc.gpsimd.dma_start(T[:, :], E[:, 0 : 2 * NCH])
            for j in range(1, 4):
                nc.gpsimd.dma_start(
                    T[:, :],
                    E[:, j * 2 * NCH : (j + 1) * 2 * NCH],
                    accum_op=ADD,
                )
            # S = T_lo + T_hi  (f32)
            s_f = r_pool.tile([128, NCH], F32, name="s_f", tag="s_f")
            nc.vector.tensor_tensor(
                s_f[:, :], T[:, 0:NCH], T[:, NCH : 2 * NCH], ADD
            )
            r_f = r_pool.tile([128, NCH], F32, name="r_f", tag="r_f")
            nc.vector.reciprocal_approx_fast(r_f[:, :], s_f[:, :])
            r_bf = r_pool.tile([128, NCH], BF16, name="r_bf", tag="r_bf")
            nc.vector.tensor_copy(r_bf[:, :], r_f[:, :])
            P = p_pool.tile([128, H * NCH], BF16, name="P", tag="P")
            nc.vector.tensor_tensor(
                P[:, :].rearrange("p (h n) -> p h n", h=H),
                E[:, :].rearrange("p (h n) -> p h n", h=H),
                r_bf[:, None, :].broadcast_to([128, H, NCH]),
                MULT,
            )
            return P

        def emit_finish(nch, psA):
            # evacuate A^T: psA[pair] partitions = e-rows 128*pair..+127
            # (split across DVE and GpSimd so neither spikes)
            a_bf = [
                a_pool.tile([128, NCH], BF16, name=f"a_bf{p}", tag="a_bf")
                for p in range(ET)
            ]
            # GpSimd cannot read PSUM; evacuation stays on DVE
            for p in range(ET):
                nc.vector.tensor_copy(a_bf[p][:, :], psA[p][:, :])
            # output projection: out[n, eo] = sum_e A^T[e, n] WoT[e, eo] + bo
            for nt2 in range(NCH // 128):
                ps_o = ps_s_pool.tile([128, D], F32, name="ps_o", tag="ps_s")
                for p in range(ET):
                    nc.tensor.matmul(
                        ps_o[:, :],
                        a_bf[p][:, nt2 * 128 : (nt2 + 1) * 128],
                        wot_bf[p][:, :],
                        start=(p == 0),
                        stop=(p == ET - 1),
                    )
                o_st = o_pool.tile([128, D], F32, name="o_st", tag="o_st")
                nc.vector.tensor_tensor(o_st[:, :], ps_o[:, :], bo_bcast[:, :], ADD)
                nc.sync.dma_start(
                    OUT[nch * NCH + nt2 * 128 : nch * NCH + (nt2 + 1) * 128, :],
                    o_st[:, :],
                )

        # deep software pipeline over all (n-chunk, m-tile) steps:
        #   step c: scores+exp(c) | norm(c-2) | A(c-4)
        # The DMA-accum head-sum chain has ~7us latency and the full norm
        # chain ~12us; lag 2 for norm and lag 4 for A absorb it. A(c-4) is
        # emitted after the scores so the PE queue never head-of-line
        # blocks the score->exp relay.
        NORM_LAG = 3
        A_LAG = 5
        steps = [(nch, mt) for nch in range(N_CHUNKS) for mt in range(MT)]
        psA_of = {}
        pend_norm = []  # [(nch, mt, E), ...]  emit_norm at lag NORM_LAG
        pend_A = []     # [(nch, mt, P), ...]  emit_A at lag A_LAG

        def step_A(rec):
            nch_, mt_, P_ = rec
            emit_A(psA_of[nch_], mt_, P_)
            if mt_ == MT - 1:
                emit_finish(nch_, psA_of[nch_])

        for ci, (nch, mt) in enumerate(steps):
            if mt == 0:
                psA_of[nch] = [
                    ps_a_pool.tile([128, NCH], F32, name=f"psA{p}", tag="psA")
                    for p in range(ET)
                ]
            E = emit_scores_exp(nch, mt)
            pend_norm.append((nch, mt, E))
            if len(pend_norm) > NORM_LAG:
                n_nch, n_mt, n_E = pend_norm.pop(0)
                pend_A.append((n_nch, n_mt, emit_norm(n_E)))
            if len(pend_A) > A_LAG - NORM_LAG:
                step_A(pend_A.pop(0))
        # drain
        while pend_norm:
            n_nch, n_mt, n_E = pend_norm.pop(0)
            pend_A.append((n_nch, n_mt, emit_norm(n_E)))
            if len(pend_A) > A_LAG - NORM_LAG:
                step_A(pend_A.pop(0))
        while pend_A:
            step_A(pend_A.pop(0))

    if repeat:
        # timing variant: loop the whole kernel on-device so execution time
        # dominates the (noisy, ~1ms) per-dispatch tunnel overhead
        with tc.For_i(0, repeat, 1):
            body()
    else:
        body()


# ---------------------------------------------------------------------------
# host wrapper

_CACHED = {}


def _get_nc():
    if "nc" not in _CACHED:
        _CACHED["nc"] = build_nc()
    return _CACHED["nc"]


def make_in_maps(Q, K, V, Wq, bq, Wo, bo):
    Q = np.asarray(Q, dtype=np.float32)
    K = np.asarray(K, dtype=np.float32)
    V = np.asarray(V, dtype=np.float32)
    WqT = np.ascontiguousarray(np.asarray(Wq, np.float32).T)
    WqTs = np.ascontiguousarray(WqT * np.float32(SCALE))
    WoT = np.ascontiguousarray(np.asarray(Wo, np.float32).T)
    bq = np.ascontiguousarray(np.asarray(bq, np.float32)).reshape(1, D)
    bqs = np.ascontiguousarray(bq * np.float32(SCALE))
    bo = np.ascontiguousarray(np.asarray(bo, np.float32)).reshape(1, D)

    in_maps = []
    for c in range(8):
        b, half = divmod(c, 2)
        in_maps.append(
            {
                "q_in": np.ascontiguousarray(Q[b, half * NQ : (half + 1) * NQ]),
                "k_in": np.ascontiguousarray(K[b]),
                "v_in": np.ascontiguousarray(V[b]),
                "wqt": WqT,
                "wqts": WqTs,
                "wot": WoT,
                "bq": bq,
                "bqs": bqs,
                "bo": bo,
            }
        )
    return in_maps


def kernel(Q, K, V, Wq, bq, Wo, bo):
    from concourse import bass_utils

    nc = _get_nc()
    in_maps = make_in_maps(Q, K, V, Wq, bq, Wo, bo)
    res = bass_utils.run_bass_kernel_spmd(nc, in_maps, core_ids=list(range(8)))

    out = np.empty((B, N, D), np.float32)
    for c in range(8):
        b, half = divmod(c, 2)
        out[b, half * NQ : (half + 1) * NQ] = res.results[c]["out"]
    return out

